# revision 16
# baseline (speedup 1.0000x reference)
"""Dense MoE (all-experts, gate-weighted sum) on 8 Trainium2 NeuronCores.

Sharding: pure data-parallel over the token axis N (8192 -> 1024 rows/core);
every core holds all 8 experts, so no collectives are needed.

Math folded per core (N_loc=1024, D=1024, E=8, O=1024, H=256):
    h      = relu(x @ W_g1.T + b_g1)                 # gating MLP, bf16 matmuls
    gates  = softmax(h @ W_g2.T + b_g2)              # fp32 softmax
    out    = sum_e gates[:,e] * (x @ W_e[e].T) + gates @ b_e

v2 schedule (vs the ~265us baseline):
  - no big dummy-warmup block: 8 tiny N=128 matmuls prime the HAM clock
    gate while the first DMAs land, then the gating GEMM itself runs and
    finishes the warmup;
  - DMA queues split: sync queue carries the gating-critical xT/W_g1
    stream then experts 1-7; the scalar queue carries expert 0's weights
    in parallel; gpsimd carries the small constants;
  - relu is emitted per psum-group so logits can start ~3us earlier, and
    expert-0 matmul groups are interleaved with the logits/softmax phase
    so the PE never idles there;
  - the 16 gate.T @ b_e bias matmuls share the main PSUM pool and are
    interleaved into expert 1's stream (the dedicated 1-buf pool used to
    serialize the PE for ~5us during expert 2);
  - expert 7's epilogue runs in half-tiles and streams the output DMA on
    the sync queue to shorten the kernel tail.

All matmul operands are bf16 (host-cast); accumulation fp32.
"""

import numpy as np
import ml_dtypes

import concourse.bass as bass
import concourse.mybir as mybir
import concourse.tile as tile
from concourse.bass_utils import run_bass_kernel_spmd

N, D, E, O, H = 8192, 1024, 8, 1024, 256
NCORES = 8
NLOC = N // NCORES          # 1024 rows per core
P = 128                     # partitions
NT = NLOC // P              # 8 n-tiles
DK = D // P                 # 8 contraction tiles
FO = 512                    # matmul moving free dim (one PSUM bank of fp32)
OH = O // FO                # 2 output halves
H2 = H // P                 # 2 h-tiles
BF16 = mybir.dt.bfloat16
F32 = mybir.dt.float32
BF = ml_dtypes.bfloat16


def legalize_single_wait(nc, max_waits=1):
    """This walrus build rejects instructions carrying more than one sync
    wait. Split each multi-wait instruction: excess waits move onto fresh
    same-engine NoOps inserted immediately before it (identical semantics:
    the engine stalls at the same program point on every semaphore)."""
    for f in nc.m.functions:
        for blk in f.blocks:
            insts = list(blk.instructions)
            if all(
                (i.sync_info is None or len(i.sync_info.on_wait) <= max_waits)
                for i in insts
            ):
                continue
            new = []
            for inst in insts:
                si = inst.sync_info
                if si is not None and len(si.on_wait) > max_waits:
                    waits = list(si.on_wait)
                    for k, w in enumerate(waits[:-max_waits]):
                        nop = mybir.InstNoOp(name=f"{inst.name}-w{k}")
                        nop.engine = inst.engine
                        nop.sync_info = mybir.SyncInfo(on_wait=[w], on_update=[])
                        new.append(nop)
                    si.on_wait = waits[-max_waits:]
                new.append(inst)
            blk.instructions = new
    return nc


def build_moe():
    nc = bass.Bass(target_bir_lowering=False)
    xT = nc.dram_tensor("xT", [D, NLOC], BF16, kind="ExternalInput")
    wt = nc.dram_tensor("wt", [E, D, O], BF16, kind="ExternalInput")
    wg1t = nc.dram_tensor("wg1t", [D, H], BF16, kind="ExternalInput")
    wg2t = nc.dram_tensor("wg2t", [H, E], BF16, kind="ExternalInput")
    bg1 = nc.dram_tensor("bg1", [H], F32, kind="ExternalInput")
    bg2 = nc.dram_tensor("bg2", [E], BF16, kind="ExternalInput")
    be = nc.dram_tensor("be", [E, O], BF16, kind="ExternalInput")
    out = nc.dram_tensor("out", [NLOC, O], F32, kind="ExternalOutput")

    with tile.TileContext(nc) as tc:
        with (
            tc.tile_pool(name="const", bufs=1) as constp,
            tc.tile_pool(name="wpool", bufs=4) as wpool,
            tc.tile_pool(name="work", bufs=4) as workp,
            tc.tile_pool(name="pro_ps", bufs=3, space="PSUM") as prop,
            tc.tile_pool(name="mm_ps", bufs=5, space="PSUM") as mmp,
        ):
            # ---- tiny PE warm-up: N=128 matmuls on memset tiles keep the
            # HAM activity window busy while the first transfers land ----
            warm_a = constp.tile([P, P], BF16, tag="warm_a")
            nc.vector.memset(warm_a, 0.0)
            warm_b = constp.tile([P, P], BF16, tag="warm_b")
            nc.vector.memset(warm_b, 0.0)
            for i in range(14):
                wpsum = mmp.tile([P, FO], F32, tag="mm", name=f"warm{i}")
                nc.tensor.matmul(
                    wpsum[:, 0:P], warm_a, warm_b, start=True, stop=True
                )

            # ---- resident inputs. The DMA fabric tops out ~350 GB/s per
            # core shared by all queues, so the two hardware queues carry
            # disjoint pieces of the critical stream: sync = xT (2MB) then
            # expert-0 weights (half 0 per-dk so expert-0 matmuls start
            # while it streams) then experts 2-7 (pool-gated); scalar =
            # wg1t then (emitted later, mid-expert-0) expert 1. ----
            wg1t_sb = [
                constp.tile([P, H], BF16, tag=f"wg1t{dk}", name=f"wg1t{dk}")
                for dk in range(DK)
            ]
            xT_sb = [
                constp.tile([P, NLOC], BF16, tag=f"xTd{dk}", name=f"xTd{dk}")
                for dk in range(DK)
            ]
            for dk in range(DK):
                nc.scalar.dma_start(
                    out=wg1t_sb[dk], in_=wg1t[dk * P : (dk + 1) * P, :]
                )
                nc.sync.dma_start(
                    out=xT_sb[dk], in_=xT[dk * P : (dk + 1) * P, :]
                )
            wt0_r = wt[0].rearrange("(dk p) o -> p dk o", p=P)
            w0_half = [
                wpool.tile([P, DK, FO], BF16, tag=f"wh{oh}", name=f"wh{oh}")
                for oh in range(OH)
            ]
            for dk in range(DK):
                nc.sync.dma_start(
                    out=w0_half[0][:, dk, :], in_=wt0_r[:, dk, 0:FO]
                )
            nc.sync.dma_start(out=w0_half[1], in_=wt0_r[:, :, FO : 2 * FO])
            wg2t_sb = constp.tile([P, H2, E], BF16, tag="wg2t")
            nc.gpsimd.dma_start(
                out=wg2t_sb, in_=wg2t.rearrange("(h2 p) e -> p h2 e", p=P)
            )
            bg1_sb = constp.tile([P, H2], F32, tag="bg1")
            nc.gpsimd.dma_start(out=bg1_sb, in_=bg1.rearrange("(h2 p) -> p h2", p=P))
            bg2_sb = constp.tile([1, E], BF16, tag="bg2")
            nc.gpsimd.dma_start(out=bg2_sb, in_=bg2[:])
            # b_e replicated at partition bases {0,32,64,96}: the bias
            # matmul stationary gates.T slices live at those bases and the
            # moving operand must share the base partition
            be_sb = constp.tile([P, O], BF16, tag="be")
            for q in range(4):
                nc.gpsimd.dma_start(
                    out=be_sb[q * 32 : q * 32 + E, :], in_=be[:, :]
                )
            ones_sb = constp.tile([1, P], BF16, tag="ones")
            nc.vector.memset(ones_sb, 1.0)

            # ---- gating: hT[h, n] = relu(W_g1 @ x.T + b_g1) ----
            # dk 0..6 interleaved across the 4 psum groups (starts as soon
            # as each dk chunk lands); dk=7 per group with relu emitted
            # immediately so hT becomes available incrementally.
            hT_sb = [
                constp.tile([P, NLOC], BF16, tag=f"hT{h2}", name=f"hT{h2}")
                for h2 in range(H2)
            ]
            NH = NLOC // FO
            groups = [(0, 0), (1, 0), (0, 1), (1, 1)]  # (h2, nh): nh=0 first
            psum_g = {
                g: mmp.tile([P, FO], F32, tag="mm", name=f"psum_g{g[0]}_{g[1]}")
                for g in groups
            }
            for dk in range(DK - 1):
                for h2, nh in groups:
                    nc.tensor.matmul(
                        psum_g[(h2, nh)],
                        wg1t_sb[dk][:, h2 * P : (h2 + 1) * P],
                        xT_sb[dk][:, nh * FO : (nh + 1) * FO],
                        start=(dk == 0),
                        stop=False,
                    )
            for h2, nh in groups:
                nc.tensor.matmul(
                    psum_g[(h2, nh)],
                    wg1t_sb[DK - 1][:, h2 * P : (h2 + 1) * P],
                    xT_sb[DK - 1][:, nh * FO : (nh + 1) * FO],
                    start=False,
                    stop=True,
                )
                nc.scalar.activation(
                    out=hT_sb[h2][:, nh * FO : (nh + 1) * FO],
                    in_=psum_g[(h2, nh)],
                    func=mybir.ActivationFunctionType.Relu,
                    bias=bg1_sb[:, h2 : h2 + 1],
                )

            # ---- gating: logits -> softmax -> gates (per nt-tile) ----
            # normalized gates are written in BF16 into "quad" tiles laid
            # out so a single 128x128 XBAR DMA-transpose per 4 nt-tiles
            # yields gates.T slices at base partitions {0,32,64,96} (legal
            # matmul stationary bases) -- no PE transposes needed.
            gates_sb = [None] * NT
            gq_in = [
                constp.tile([P, P], BF16, tag=f"gq_in{h}", name=f"gq_in{h}")
                for h in range(2)
            ]
            gq_out = [
                constp.tile([P, P], BF16, tag=f"gq_out{h}", name=f"gq_out{h}")
                for h in range(2)
            ]

            def emit_logits_softmax(nt):
                psum_l = prop.tile([P, E], F32, tag="pro")
                for h2 in range(H2):
                    nc.tensor.matmul(
                        psum_l,
                        hT_sb[h2][:, nt * P : (nt + 1) * P],
                        wg2t_sb[:, h2, :],
                        start=(h2 == 0),
                        stop=False,
                    )
                nc.tensor.matmul(psum_l, ones_sb, bg2_sb, start=False, stop=True)
                negmax = workp.tile([P, 1], F32, tag="negmax")
                nc.vector.reduce_max(
                    negmax, psum_l, axis=mybir.AxisListType.X, negate=True
                )
                gates = constp.tile([P, E], F32, tag=f"gates{nt}", name=f"gates{nt}")
                sumexp = workp.tile([P, 1], F32, tag="sumexp")
                nc.scalar.activation(
                    out=gates,
                    in_=psum_l,
                    func=mybir.ActivationFunctionType.Exp,
                    bias=negmax,
                    accum_out=sumexp,
                )
                rsum = workp.tile([P, 1], F32, tag="rsum")
                nc.vector.reciprocal(rsum, sumexp)
                slot = (nt % 4) * 32
                gbf = gq_in[nt // 4][:, slot : slot + E]
                nc.vector.tensor_scalar_mul(gbf, gates, rsum)
                nc.vector.tensor_scalar_mul(gates, gates, rsum)
                gates_sb[nt] = gates

            acc_sb = [
                [
                    constp.tile(
                        [P, FO], F32, tag=f"acc{nt}_{oh}", name=f"acc{nt}_{oh}"
                    )
                    for oh in range(OH)
                ]
                for nt in range(NT)
            ]

            # ---- expert matmul groups ----
            def emit_expert_group_mms(w_half, oh, nt):
                psum = mmp.tile([P, FO], F32, tag="mm")
                for dk in range(DK):
                    nc.tensor.matmul(
                        psum,
                        xT_sb[dk][:, nt * P : (nt + 1) * P],
                        w_half[oh][:, dk, :],
                        start=(dk == 0),
                        stop=(dk == DK - 1),
                    )
                return psum

            def emit_mul(psum, e, oh, nt):
                acc = acc_sb[nt][oh]
                if e == 0:
                    nc.scalar.mul(acc, psum, gates_sb[nt][:, e : e + 1])
                else:
                    tmp = workp.tile([P, FO], F32, tag="tmp", name="tmp")
                    nc.scalar.mul(tmp, psum, gates_sb[nt][:, e : e + 1])
                    nc.vector.tensor_add(acc, acc, tmp)

            # expert-0 runs in 4-group batches with dk OUTER so the PE can
            # consume expert-0 weight chunks while they are still landing,
            # interleaved with logits/softmax. ACT-FIFO order stays:
            # relu x4 -> exp nt0-3 -> exp nt4-7 -> gate-muls (no deadlock).
            def emit_batch_dks(psums, oh, nts, dks):
                for dk in dks:
                    for i, nt in enumerate(nts):
                        nc.tensor.matmul(
                            psums[i],
                            xT_sb[dk][:, nt * P : (nt + 1) * P],
                            w0_half[oh][:, dk, :],
                            start=(dk == 0),
                            stop=(dk == DK - 1),
                        )

            b1 = [mmp.tile([P, FO], F32, tag="mm", name=f"b1_{i}") for i in range(4)]
            emit_batch_dks(b1, 0, range(4), range(0, 2))
            for nt in range(4):
                emit_logits_softmax(nt)
            emit_batch_dks(b1, 0, range(4), range(2, 5))
            for nt in range(4, NT):
                emit_logits_softmax(nt)
            emit_batch_dks(b1, 0, range(4), range(5, DK))
            for i, nt in enumerate(range(4)):
                emit_mul(b1[i], 0, 0, nt)
            b2 = [mmp.tile([P, FO], F32, tag="mm", name=f"b2_{i}") for i in range(4)]
            emit_batch_dks(b2, 0, range(4, NT), range(DK))
            for i, nt in enumerate(range(4, NT)):
                emit_mul(b2[i], 0, 0, nt)
            # gates.T via XBAR DMA-transpose on the scalar queue (runs
            # mid-expert-0, far ahead of the expert-1 bias matmuls), and
            # expert-1's weight DMAs emitted here so the scalar queue only
            # starts pulling them after the startup-critical window.
            for h in range(2):
                nc.scalar.dma_start_transpose(out=gq_out[h], in_=gq_in[h])
            wt1_r = wt[1].rearrange("(dk p) o -> p dk o", p=P)
            w1_half = []
            for oh in range(OH):
                wh = wpool.tile([P, DK, FO], BF16, tag=f"wh{oh}", name=f"wh{oh}")
                nc.scalar.dma_start(
                    out=wh, in_=wt1_r[:, :, oh * FO : (oh + 1) * FO]
                )
                w1_half.append(wh)

            def gatesT_ap(nt):
                base = (nt % 4) * 32
                return gq_out[nt // 4][base : base + E, :]

            for nt in range(NT):
                psum = emit_expert_group_mms(w0_half, 1, nt)
                emit_mul(psum, 0, 1, nt)

            # ---- experts 1-7: expert 1's weights arrived on the scalar
            # queue; 2-7 stream on sync (pool-gated). bias (gates.T @ b_e)
            # pairs (both oh halves, shared stationary) interleave into
            # expert 1's oh==0 stream with psums from the prop pool. ----
            for e in range(1, E):
                if e == 1:
                    w_half = w1_half
                else:
                    wt_r = wt[e].rearrange("(dk p) o -> p dk o", p=P)
                    w_half = []
                    for oh in range(OH):
                        wh = wpool.tile(
                            [P, DK, FO], BF16, tag=f"wh{oh}", name=f"wh{oh}"
                        )
                        nc.sync.dma_start(
                            out=wh, in_=wt_r[:, :, oh * FO : (oh + 1) * FO]
                        )
                        w_half.append(wh)
                for oh in range(OH):
                    for nt in range(NT):
                        last_grp = e == E - 1 and oh == OH - 1 and nt == NT - 1
                        if last_grp:
                            # very last group: two independent half-width
                            # psum accumulations so the first half's
                            # epilogue+DMA overlaps the second half's
                            # matmuls -- shortens the kernel tail
                            psum = mmp.tile([P, FO], F32, tag="mm")
                            acc = acc_sb[nt][oh]
                            tmp = workp.tile([P, FO], F32, tag="tmp", name="tmp")
                            for hh in range(2):
                                sl = slice(hh * (FO // 2), (hh + 1) * (FO // 2))
                                for dk in range(DK):
                                    nc.tensor.matmul(
                                        psum[:, sl],
                                        xT_sb[dk][:, nt * P : (nt + 1) * P],
                                        w_half[oh][
                                            :, dk, hh * (FO // 2) : (hh + 1) * (FO // 2)
                                        ],
                                        start=(dk == 0),
                                        stop=(dk == DK - 1),
                                    )
                                nc.scalar.mul(
                                    tmp[:, sl], psum[:, sl],
                                    gates_sb[nt][:, e : e + 1],
                                )
                                nc.vector.tensor_add(
                                    acc[:, sl], acc[:, sl], tmp[:, sl]
                                )
                                nc.sync.dma_start(
                                    out=out[
                                        nt * P : (nt + 1) * P,
                                        oh * FO + hh * (FO // 2)
                                        : oh * FO + (hh + 1) * (FO // 2),
                                    ],
                                    in_=acc[:, sl],
                                )
                            continue
                        psum = emit_expert_group_mms(w_half, oh, nt)
                        if e == E - 1:
                            # final expert: half-tile epilogue + streamed
                            # output DMA to shorten the kernel tail
                            acc = acc_sb[nt][oh]
                            tmp = workp.tile([P, FO], F32, tag="tmp", name="tmp")
                            for hh in range(2):
                                sl = slice(hh * (FO // 2), (hh + 1) * (FO // 2))
                                nc.scalar.mul(
                                    tmp[:, sl], psum[:, sl],
                                    gates_sb[nt][:, e : e + 1],
                                )
                                nc.vector.tensor_add(
                                    acc[:, sl], acc[:, sl], tmp[:, sl]
                                )
                                nc.sync.dma_start(
                                    out=out[
                                        nt * P : (nt + 1) * P,
                                        oh * FO + hh * (FO // 2)
                                        : oh * FO + (hh + 1) * (FO // 2),
                                    ],
                                    in_=acc[:, sl],
                                )
                        else:
                            emit_mul(psum, e, oh, nt)
                        if e == 1 and oh == 0:
                            # bias pair for this nt: both output halves,
                            # stationary gates.T loaded once
                            base = (nt % 4) * 32
                            for boh in range(OH):
                                psum_b = prop.tile(
                                    [P, FO], F32, tag="pro", name="psum_b"
                                )
                                nc.tensor.matmul(
                                    psum_b,
                                    gatesT_ap(nt),
                                    be_sb[
                                        base : base + E,
                                        boh * FO : (boh + 1) * FO,
                                    ],
                                    start=True,
                                    stop=True,
                                    tile_position=(base, 0),
                                )
                                nc.vector.tensor_add(
                                    acc_sb[nt][boh], acc_sb[nt][boh], psum_b
                                )

    legalize_single_wait(nc)
    return nc


_NC_CACHE = {}


def _get_nc():
    if "nc" not in _NC_CACHE:
        _NC_CACHE["nc"] = build_moe()
    return _NC_CACHE["nc"]


def make_in_maps(x, W_e, b_e, W_g1, b_g1, W_g2, b_g2):
    x = np.asarray(x, dtype=np.float32)
    wt = np.ascontiguousarray(
        np.asarray(W_e, dtype=np.float32).transpose(0, 2, 1)
    ).astype(BF)
    wg1t = np.ascontiguousarray(np.asarray(W_g1, dtype=np.float32).T).astype(BF)
    wg2t = np.ascontiguousarray(np.asarray(W_g2, dtype=np.float32).T).astype(BF)
    bg1 = np.asarray(b_g1, dtype=np.float32)
    bg2 = np.asarray(b_g2, dtype=np.float32).astype(BF)
    be = np.asarray(b_e, dtype=np.float32).astype(BF)
    xb = x.astype(BF)
    in_maps = []
    for c in range(NCORES):
        xT_c = np.ascontiguousarray(xb[c * NLOC : (c + 1) * NLOC, :].T)
        in_maps.append(
            {
                "xT": xT_c,
                "wt": wt,
                "wg1t": wg1t,
                "wg2t": wg2t,
                "bg1": bg1,
                "bg2": bg2,
                "be": be,
            }
        )
    return in_maps


def kernel(x, W_e, b_e, W_g1, b_g1, W_g2, b_g2, **run_kwargs):
    nc = _get_nc()
    in_maps = make_in_maps(x, W_e, b_e, W_g1, b_g1, W_g2, b_g2)
    res = run_bass_kernel_spmd(nc, in_maps, core_ids=list(range(NCORES)), **run_kwargs)
    out = np.concatenate([res.results[c]["out"] for c in range(NCORES)], axis=0)
    if run_kwargs:
        kernel.last_results = res
    return out


if __name__ == "__main__":
    rng = np.random.default_rng(0)
    s = 1.0 / np.sqrt(D)
    sh = 1.0 / np.sqrt(H)
    inputs = {
        "x": rng.standard_normal((N, D), dtype=np.float32),
        "W_e": rng.uniform(-s, s, (E, O, D)).astype(np.float32),
        "b_e": rng.uniform(-s, s, (E, O)).astype(np.float32),
        "W_g1": rng.uniform(-s, s, (H, D)).astype(np.float32),
        "b_g1": rng.uniform(-sh, sh, (H,)).astype(np.float32),
        "W_g2": rng.uniform(-sh, sh, (E, H)).astype(np.float32),
        "b_g2": rng.uniform(-sh, sh, (E,)).astype(np.float32),
    }
    out = kernel(**inputs)
    print("out", out.shape, out.dtype, float(np.abs(out).max()))
